# revision 27
# baseline (speedup 1.0000x reference)
"""LRU (Linear Recurrent Unit) Bass kernel for Trainium2, 8 NeuronCores.

v4.1: chunked-weights formulation. All per-timestep complex rotations are
folded into K per-offset weight matrices (u = t mod K), so the PE does
them for free. Data lives in u-major layout (pos = u*NC + c):

  bt'_u = (diag(e^{-i theta u}) B~)^T x_u      (PE, K weight sets)
  m_u   = r (.) m_{u-1} + bt'_u                (DVE fused stt per u-row)
  chunk carry: W_c = lambda^K W_{c-1} + P_c    (small complex scan, NC cols)
  w_u   = m_u + r^{u+1} (.) V,  V_c = e^{i phi c} G_{c-1}
  y     = ReC^_u w_re - ImC^_u w_im            (PE, K weight sets)

The w-correction is split across engines per u: DVE (fused stt), Pool
(two tensor-tensor ops with broadcast r^{u+1} tiles), or PE (extra
matmuls against V with r^{u+1}-scaled C weights, accumulated in the same
PSUM as the main output matmuls).

Host pre/post: u-major permutation of x and y, weight/table precompute.
"""

import sys

for _p in ("/opt/trn_rl_repo", "/root/.axon_site/_ro/trn_rl_repo"):
    if _p not in sys.path:
        sys.path.append(_p)

import numpy as np

N = 256
T = 8192
BATCH = 8
NCORES = 8
K = 16          # chunk length (weight-set count)
NC = T // K     # number of chunks = columns per u-row
CS = 2          # c-slabs for pipelining
NCs = NC // CS

U_DVE = (0, 1, 2, 3, 4, 6, 8, 10, 12, 14)
U_POOL = ()
U_PE = (5, 7, 9, 11, 13, 15)

_cache = {}


def _build_program():
    from concourse import bacc, tile
    from concourse import mybir

    fp32 = mybir.dt.float32
    fp16 = mybir.dt.float16
    Copy = mybir.ActivationFunctionType.Copy
    mult = mybir.AluOpType.mult
    add = mybir.AluOpType.add

    nc = bacc.Bacc(None, target_bir_lowering=False, debug=False)

    # DRAM parameters (per core) - packed for few, large DMAs
    NW2 = len(U_PE) * 4
    xall = nc.declare_dram_parameter("xall", [128, 2 * T], fp16,
                                     isOutput=False)
    wBall = nc.declare_dram_parameter("wBall", [128, 64 * N], fp16,
                                      isOutput=False)
    wCall = nc.declare_dram_parameter("wCall", [128, (64 + NW2) * N], fp16,
                                      isOutput=False)
    tab16 = nc.declare_dram_parameter("tab16", [N, 3 * NC], fp16,
                                      isOutput=False)
    tab32 = nc.declare_dram_parameter("tab32", [N, 1 + K], fp32,
                                      isOutput=False)
    outy = nc.declare_dram_parameter("outy", [N, T], fp16, isOutput=True)

    PL = ("re", "im")
    # wCall col-block index for wc(u,p,nh): (u*2+p)*2+nh ; wc2(i,p,nh): 64+...
    UE_E = tuple(u for u in U_PE if u < 10)   # early wc2 (in wcE pool)
    UE_L = tuple(u for u in U_PE if u >= 10)  # late wc2 (in wcL pool)

    with tile.TileContext(nc) as tc:
        with (
            tc.tile_pool(name="const", bufs=1) as cpool,
            tc.tile_pool(name="m", bufs=1) as mpool,
            tc.tile_pool(name="ck", bufs=1) as ckpool,
            tc.tile_pool(name="bp", bufs=1) as bpool,
            tc.tile_pool(name="w", bufs=2) as wpool,
            tc.tile_pool(name="ys", bufs=3) as ypool,
            tc.tile_pool(name="pa", bufs=1, space="PSUM") as papool,
            tc.tile_pool(name="py", bufs=2, space="PSUM") as pypool,
        ):
            # ---- persistent small tables ----
            cosT, sinT, rkbT, rcolT, rseedT = {}, {}, {}, {}, {}
            tabdma = []
            for mh in range(2):
                t16 = cpool.tile([128, 3 * NC], fp16, name=f"t16_{mh}",
                                 tag=f"t16_{mh}")
                cosT[mh] = t16[:, 0:NC]
                sinT[mh] = t16[:, NC:2 * NC]
                rkbT[mh] = t16[:, 2 * NC:3 * NC]
                t32 = cpool.tile([128, 1 + K], fp32, name=f"t32_{mh}",
                                 tag=f"t32_{mh}")
                rcolT[mh] = t32[:, 0:1]
                rseedT[mh] = t32[:, 1:1 + K]
                tabdma.append((t16, t32, mh))

            m = {}
            for u in range(K):
                for pl in PL:
                    for mh in range(2):
                        m[(u, pl, mh)] = mpool.tile(
                            [128, NC], fp16, name=f"m{u}{pl}{mh}",
                            tag=f"m{u}{pl}{mh}")
            g, V = {}, {}
            for pl in PL:
                for mh in range(2):
                    gt = ckpool.tile([128, NC + 1], fp16, name=f"g{pl}{mh}",
                                     tag=f"g{pl}{mh}")
                    nc.gpsimd.memset(gt[:, 0:1], 0.0)
                    g[(pl, mh)] = gt
                    V[(pl, mh)] = ckpool.tile([128, NC], fp16,
                                              name=f"V{pl}{mh}",
                                              tag=f"V{pl}{mh}")

            wc, wc2 = {}, {}

            # =========== phase 1: A (whole NC), with x+wB pools ===========
            wcE_cm = tc.tile_pool(name="wcE", bufs=1)
            wcE = wcE_cm.__enter__()
            with tc.tile_pool(name="xw", bufs=1) as xwpool:
                xbig = xwpool.tile([128, 2 * T], fp16, name="xbig",
                                   tag="xbig")
                wbbig = xwpool.tile([128, 64 * N], fp16, name="wbbig",
                                    tag="wbbig")
                # early wc pool: wc u<10 + wc2 for UE_E
                wcEt = wcE.tile([128, (40 + 4 * len(UE_E)) * N], fp16,
                                name="wcE", tag="wcE")
                for u in range(10):
                    for p in range(2):
                        for nh in range(2):
                            b0 = ((u * 2 + p) * 2 + nh) * N
                            wc[(u, PL[p], nh)] = wcEt[:, b0:b0 + N]
                for j, u in enumerate(UE_E):
                    for p in range(2):
                        for nh in range(2):
                            b0 = (40 + (j * 2 + p) * 2 + nh) * N
                            wc2[(u, PL[p], nh)] = wcEt[:, b0:b0 + N]

                # fp32 tables first (tiny; the chain stt needs rcolT)
                for t16, t32, mh in tabdma:
                    sl = slice(mh * 128, (mh + 1) * 128)
                    nc.sync.dma_start(out=t32[:], in_=tab32[sl, :])
                # DMA pieces ordered by consumption (first pieces 1-u)
                for up0, up1 in ((0, 1), (1, 2)) + tuple(
                        (i, i + 2) for i in range(2, K, 2)):
                    c0, c1 = up0 * 4 * N, up1 * 4 * N
                    nc.sync.dma_start(out=wbbig[:, c0:c1],
                                      in_=wBall[:, c0:c1])
                    for nh in range(2):
                        x0 = nh * T + up0 * NC
                        x1 = nh * T + up1 * NC
                        nc.sync.dma_start(out=xbig[:, x0:x1],
                                          in_=xall[:, x0:x1])
                    up = up0
                    if up == 2:
                        for t16, _t32, mh in tabdma:
                            sl = slice(mh * 128, (mh + 1) * 128)
                            nc.sync.dma_start(out=t16[:], in_=tab16[sl, :])
                # early wc: wc u<10 blocks are wCall cols [0:40N); wc2 blocks
                # for UE_E start at (64 + idx(U_PE)*4)*N
                nc.sync.dma_start(out=wcEt[:, 0:20 * N],
                                  in_=wCall[:, 0:20 * N])
                nc.sync.dma_start(out=wcEt[:, 20 * N:40 * N],
                                  in_=wCall[:, 20 * N:40 * N])
                if UE_E:
                    i0 = U_PE.index(UE_E[0])
                    s0, s1 = (64 + i0 * 4) * N, (64 + (i0 + len(UE_E)) * 4) * N
                    half = 40 * N + (s1 - s0) // 2
                    nc.sync.dma_start(out=wcEt[:, 40 * N:half],
                                      in_=wCall[:, s0:s0 + (s1 - s0) // 2])
                    nc.sync.dma_start(out=wcEt[:, half:],
                                      in_=wCall[:, s0 + (s1 - s0) // 2:s1])

                # ---- Phase A: whole-NC rows ----
                for u in range(K):
                    ps = {}
                    for pl in PL:
                        p = PL.index(pl)
                        for mh in range(2):
                            p_ = papool.tile([128, NC], fp32,
                                             name=f"pa{pl}{mh}",
                                             tag=f"pa{pl}{mh}")
                            for nh in range(2):
                                b0 = ((u * 2 + p) * 2 + nh) * N
                                nc.tensor.matmul(
                                    p_[:],
                                    wbbig[:, b0 + mh * 128:b0 + mh * 128 + 128],
                                    xbig[:, nh * T + u * NC:
                                         nh * T + (u + 1) * NC],
                                    start=(nh == 0), stop=(nh == 1),
                                )
                            ps[(pl, mh)] = p_
                    for pl in PL:
                        for mh in range(2):
                            dst = m[(u, pl, mh)][:]
                            if u == 0:
                                nc.scalar.activation(dst, ps[(pl, mh)][:],
                                                     Copy)
                            else:
                                nc.vector.scalar_tensor_tensor(
                                    dst, m[(u - 1, pl, mh)][:],
                                    rcolT[mh][:, 0:1], ps[(pl, mh)][:],
                                    mult, add,
                                )

            # =========== phase 2: B + C, slabbed ===========
            with tc.tile_pool(name="wcL", bufs=1) as wcL:
                pass_marker = None
                wcLt = wcL.tile([128, (24 + 4 * len(UE_L)) * N], fp16,
                                name="wcL", tag="wcL")
                for u in range(10, K):
                    for p in range(2):
                        for nh in range(2):
                            b0 = ((u - 10) * 4 + p * 2 + nh) * N
                            wc[(u, PL[p], nh)] = wcLt[:, b0:b0 + N]
                for j, u in enumerate(UE_L):
                    for p in range(2):
                        for nh in range(2):
                            b0 = (24 + (j * 2 + p) * 2 + nh) * N
                            wc2[(u, PL[p], nh)] = wcLt[:, b0:b0 + N]
                # late wc DMAs (hidden under B0 / early C)
                nc.sync.dma_start(out=wcLt[:, 0:12 * N],
                                  in_=wCall[:, 40 * N:52 * N])
                nc.sync.dma_start(out=wcLt[:, 12 * N:24 * N],
                                  in_=wCall[:, 52 * N:64 * N])
                if UE_L:
                    i0 = U_PE.index(UE_L[0])
                    s0 = (64 + i0 * 4) * N
                    s1 = (64 + (i0 + len(UE_L)) * 4) * N
                    nc.sync.dma_start(out=wcLt[:, 24 * N:],
                                      in_=wCall[:, s0:s1])

                ones = wcL.tile([128, NC], fp16, name="ones", tag="ones")
                nc.gpsimd.memset(ones[:], 1.0)
                rsb = {}
                for u in U_POOL:
                    for mh in range(2):
                        t_ = wcL.tile([128, NC], fp16, name=f"rsb{u}{mh}",
                                      tag=f"rsb{u}{mh}")
                        nc.scalar.activation(t_[:], ones[:], Copy,
                                             scale=rseedT[mh][:, u:u + 1])
                        rsb[(u, mh)] = t_

                def phase_b(s):
                    lo, hi = s * NCs, (s + 1) * NCs
                    for mh in range(2):
                        pre = m[(K - 1, "re", mh)][:, lo:hi]
                        pim = m[(K - 1, "im", mh)][:, lo:hi]
                        cs_ = cosT[mh][:, lo:hi]
                        sn_ = sinT[mh][:, lo:hi]
                        a = bpool.tile([128, NCs], fp16, name=f"ba{mh}",
                                       tag=f"ba{mh}")
                        b = bpool.tile([128, NCs], fp16, name=f"bb{mh}",
                                       tag=f"bb{mh}")
                        pr = bpool.tile([128, NCs], fp16, name=f"pr{mh}",
                                        tag=f"pr{mh}")
                        pi = bpool.tile([128, NCs], fp16, name=f"pi{mh}",
                                        tag=f"pi{mh}")
                        nc.vector.tensor_mul(a[:], cs_, pre)
                        nc.vector.tensor_mul(b[:], sn_, pim)
                        nc.vector.tensor_add(pr[:], a[:], b[:])
                        nc.vector.tensor_mul(a[:], cs_, pim)
                        nc.vector.tensor_mul(b[:], sn_, pre)
                        nc.vector.tensor_sub(pi[:], a[:], b[:])
                        for pl, srct in (("re", pr), ("im", pi)):
                            gt = g[(pl, mh)]
                            nc.vector.tensor_tensor_scan(
                                gt[:, lo + 1:hi + 1], rkbT[mh][:, lo:hi],
                                srct[:], gt[:, lo:lo + 1], mult, add,
                            )
                        gre = g[("re", mh)][:, lo:hi]
                        gim = g[("im", mh)][:, lo:hi]
                        nc.vector.tensor_mul(a[:], cs_, gre)
                        nc.vector.tensor_mul(b[:], sn_, gim)
                        nc.vector.tensor_sub(V[("re", mh)][:, lo:hi],
                                             a[:], b[:])
                        nc.vector.tensor_mul(a[:], cs_, gim)
                        nc.vector.tensor_mul(b[:], sn_, gre)
                        nc.vector.tensor_add(V[("im", mh)][:, lo:hi],
                                             a[:], b[:])

                yq = {}

                pyheld = {}

                def prefill_c_u(s, u):
                    # m-part matmuls for a U_PE u; psum held until finish
                    lo, hi = s * NCs, (s + 1) * NCs
                    for mo in range(2):
                        p_ = pypool.tile([128, NCs], fp32, name=f"py{mo}",
                                         tag=f"py{mo}")
                        kk = 0
                        for pl in PL:
                            for mh in range(2):
                                wcs = wc[(u, pl, mh)]
                                nc.tensor.matmul(
                                    p_[:],
                                    wcs[:, mo * 128:(mo + 1) * 128],
                                    m[(u, pl, mh)][:, lo:hi],
                                    start=(kk == 0), stop=False,
                                )
                                kk += 1
                        pyheld[(s, u, mo)] = p_

                def finish_c_u(s, u):
                    lo, hi = s * NCs, (s + 1) * NCs
                    for mo in range(2):
                        p_ = pyheld.pop((s, u, mo))
                        kk = 4
                        for pl in PL:
                            for mh in range(2):
                                w2s = wc2[(u, pl, mh)]
                                nc.tensor.matmul(
                                    p_[:],
                                    w2s[:, mo * 128:(mo + 1) * 128],
                                    V[(pl, mh)][:, lo:hi],
                                    start=False, stop=(kk == 7),
                                )
                                kk += 1
                        _evict_y(s, u, mo, p_)

                def _evict_y(s, u, mo, p_):
                    q, uq = u // 4, u % 4
                    ent = yq.get((s, mo, q))
                    if ent is None:
                        ent = [ypool.tile([128, 4 * NCs], fp16,
                                          name=f"y{mo}", tag=f"y{mo}"), 0]
                        yq[(s, mo, q)] = ent
                    yt = ent[0]
                    nc.scalar.activation(yt[:, uq * NCs:(uq + 1) * NCs],
                                         p_[:], Copy)
                    ent[1] += 1
                    if ent[1] == 4:
                        base = s * T // CS + q * 4 * NCs
                        nc.scalar.dma_start(
                            out=outy[mo * 128:(mo + 1) * 128,
                                     base:base + 4 * NCs],
                            in_=yt[:],
                        )
                        del yq[(s, mo, q)]

                def phase_c_u(s, u):
                    lo, hi = s * NCs, (s + 1) * NCs
                    w_ = {}
                    if u in U_PE:
                        for pl in PL:
                            for mh in range(2):
                                w_[(pl, mh)] = m[(u, pl, mh)][:, lo:hi]
                    else:
                        for pl in PL:
                            for mh in range(2):
                                t_ = wpool.tile([128, NCs], fp16,
                                                name=f"w{pl}{mh}",
                                                tag=f"w{pl}{mh}")
                                if u in U_POOL:
                                    t2 = wcL.tile([128, NCs], fp16,
                                                  name=f"vt{pl}{mh}",
                                                  tag=f"vt{pl}{mh}")
                                    nc.gpsimd.tensor_mul(
                                        t2[:], rsb[(u, mh)][:, lo:hi],
                                        V[(pl, mh)][:, lo:hi])
                                    nc.gpsimd.tensor_add(
                                        t_[:], t2[:],
                                        m[(u, pl, mh)][:, lo:hi])
                                else:
                                    nc.vector.scalar_tensor_tensor(
                                        t_[:], V[(pl, mh)][:, lo:hi],
                                        rseedT[mh][:, u:u + 1],
                                        m[(u, pl, mh)][:, lo:hi],
                                        mult, add,
                                    )
                                w_[(pl, mh)] = t_
                    for mo in range(2):
                        p_ = pypool.tile([128, NCs], fp32, name=f"py{mo}",
                                         tag=f"py{mo}")
                        nmm = 8 if u in U_PE else 4
                        kk = 0
                        for pl in PL:
                            p = PL.index(pl)
                            for mh in range(2):
                                wcs = wc[(u, pl, mh)]
                                nc.tensor.matmul(
                                    p_[:],
                                    wcs[:, mo * 128:(mo + 1) * 128],
                                    w_[(pl, mh)][:],
                                    start=(kk == 0), stop=(kk == nmm - 1),
                                )
                                kk += 1
                        if u in U_PE:
                            for pl in PL:
                                for mh in range(2):
                                    w2s = wc2[(u, pl, mh)]
                                    nc.tensor.matmul(
                                        p_[:],
                                        w2s[:, mo * 128:(mo + 1) * 128],
                                        V[(pl, mh)][:, lo:hi],
                                        start=False, stop=(kk == nmm - 1),
                                    )
                                    kk += 1
                        _evict_y(s, u, mo, p_)

                c_order = (9, 0, 1, 11, 2, 3, 4, 6, 8,
                           10, 12, 14, 13, 15)
                pre0 = (5, 7)
                for u in pre0:
                    prefill_c_u(0, u)
                phase_b(0)
                for u in pre0:
                    finish_c_u(0, u)
                for u in c_order:
                    if u not in pre0:
                        phase_c_u(0, u)
                for u in pre0:
                    prefill_c_u(1, u)
                phase_b(1)
                for u in pre0:
                    finish_c_u(1, u)
                for u in c_order:
                    if u not in pre0:
                        phase_c_u(1, u)
            wcE_cm.__exit__(None, None, None)

    nc.compile()
    return nc


def _host_prep(x, nu_log, theta_log, gamma_log, B_re, B_im, C_re, C_im):
    f64 = np.float64
    r = np.exp(-np.exp(nu_log.astype(f64)))            # [N]
    theta = np.exp(theta_log.astype(f64))              # [N]
    gamma = np.exp(gamma_log.astype(f64))              # [N]
    Bre = B_re.astype(f64) * gamma[:, None]            # [N,N] rows=out m
    Bim = B_im.astype(f64) * gamma[:, None]
    Cre = C_re.astype(f64)
    Cim = C_im.astype(f64)

    u = np.arange(K, dtype=f64)
    cu, su = np.cos(theta[:, None] * u), np.sin(theta[:, None] * u)  # [N,K]

    wBs = np.empty((K * 2 * N, N), dtype=f64)
    wCs = np.empty((K * 2 * N, N), dtype=f64)
    wC2s = np.empty((len(U_PE) * 2 * N, N), dtype=f64)
    rpow = r[:, None] ** (u[None, :] + 1.0)            # [N,K]
    for uu in range(K):
        c_, s_ = cu[:, uu], su[:, uu]
        reB = c_[:, None] * Bre + s_[:, None] * Bim
        imB = c_[:, None] * Bim - s_[:, None] * Bre
        wBs[(2 * uu) * N:(2 * uu + 1) * N] = reB.T
        wBs[(2 * uu + 1) * N:(2 * uu + 2) * N] = imB.T
        reC = (c_[None, :] * Cre - s_[None, :] * Cim).T
        imC = -(s_[None, :] * Cre + c_[None, :] * Cim).T
        wCs[(2 * uu) * N:(2 * uu + 1) * N] = reC
        wCs[(2 * uu + 1) * N:(2 * uu + 2) * N] = imC
        if uu in U_PE:
            i = U_PE.index(uu)
            sc = rpow[:, uu][:, None]
            wC2s[(2 * i) * N:(2 * i + 1) * N] = sc * reC
            wC2s[(2 * i + 1) * N:(2 * i + 2) * N] = sc * imC

    phi = K * theta
    c_ = np.arange(NC, dtype=f64)
    ang = phi[:, None] * c_[None, :]

    def pack_w(ws):
        # rows [(u*2+p)*N + nh*128 : +128] -> col block ((u*2+p)*2+nh)*N
        nb = ws.shape[0] // 128
        return np.concatenate([ws[i * 128:(i + 1) * 128, :]
                               for i in range(nb)], axis=1)

    cosv = np.cos(ang)
    sinv = np.sin(ang)
    rkv = np.broadcast_to((r ** K)[:, None], (N, NC))
    shared = {
        "wBall": pack_w(wBs).astype(np.float16),
        "wCall": np.concatenate(
            [pack_w(wCs), pack_w(wC2s)], axis=1).astype(np.float16),
        "tab16": np.concatenate([cosv, sinv, rkv], axis=1).astype(np.float16),
        "tab32": np.concatenate([r[:, None], rpow], axis=1).astype(np.float32),
    }
    # u-major x: xu[:, u*NC + c] = xT[:, c*K + u]; both nh halves packed
    xus = []
    for b in range(BATCH):
        xT = np.ascontiguousarray(x[b].T).astype(np.float16)   # [N, T]
        xum = xT.reshape(N, NC, K).transpose(0, 2, 1).reshape(N, T)
        xa = np.concatenate([xum[0:128, :], xum[128:256, :]], axis=1)
        xus.append(np.ascontiguousarray(xa))
    return shared, xus


def kernel(x, nu_log, theta_log, gamma_log, B_re, B_im, C_re, C_im,
           _want_trace=False):
    from concourse import bass_utils

    x = np.asarray(x)
    nu_log = np.asarray(nu_log)
    theta_log = np.asarray(theta_log)
    gamma_log = np.asarray(gamma_log)
    B_re, B_im = np.asarray(B_re), np.asarray(B_im)
    C_re, C_im = np.asarray(C_re), np.asarray(C_im)

    if "nc" not in _cache:
        _cache["nc"] = _build_program()
    nc = _cache["nc"]

    shared, xus = _host_prep(x, nu_log, theta_log, gamma_log,
                             B_re, B_im, C_re, C_im)
    in_maps = [dict(shared, xall=xus[i]) for i in range(NCORES)]
    import os
    os.environ["BASS_NEVER_TRACE"] = "1"
    res = bass_utils.run_bass_kernel_spmd(
        nc, in_maps, core_ids=list(range(NCORES)), trace=False,
    )
    _cache["last_result"] = res
    outs = []
    for i in range(NCORES):
        # layout: [n, s, q, uq, coff]; t = (s*NCs+coff)*K + (q*4+uq)
        yu = res.results[i]["outy"]
        yT = (yu.reshape(N, CS, K // 4, 4, NCs)
              .transpose(0, 1, 4, 2, 3).reshape(N, T))
        outs.append(yT.T)
    return np.stack(outs).astype(np.float32)


if __name__ == "__main__":
    rng = np.random.default_rng(0)
    ins = {
        "x": rng.standard_normal((BATCH, T, N), dtype=np.float32),
        "nu_log": rng.standard_normal(N).astype(np.float32),
        "theta_log": rng.standard_normal(N).astype(np.float32),
        "gamma_log": rng.standard_normal(N).astype(np.float32),
        "B_re": rng.standard_normal((N, N), dtype=np.float32) * 0.04,
        "B_im": rng.standard_normal((N, N), dtype=np.float32) * 0.04,
        "C_re": rng.standard_normal((N, N), dtype=np.float32) * 0.06,
        "C_im": rng.standard_normal((N, N), dtype=np.float32) * 0.06,
    }
    out = kernel(**ins)
    print("out", out.shape, out.dtype, np.abs(out).max())


# revision 28
# speedup vs baseline: 1.0054x; 1.0054x over previous
"""LRU (Linear Recurrent Unit) Bass kernel for Trainium2, 8 NeuronCores.

v4.1: chunked-weights formulation. All per-timestep complex rotations are
folded into K per-offset weight matrices (u = t mod K), so the PE does
them for free. Data lives in u-major layout (pos = u*NC + c):

  bt'_u = (diag(e^{-i theta u}) B~)^T x_u      (PE, K weight sets)
  m_u   = r (.) m_{u-1} + bt'_u                (DVE fused stt per u-row)
  chunk carry: W_c = lambda^K W_{c-1} + P_c    (small complex scan, NC cols)
  w_u   = m_u + r^{u+1} (.) V,  V_c = e^{i phi c} G_{c-1}
  y     = ReC^_u w_re - ImC^_u w_im            (PE, K weight sets)

The w-correction is split across engines per u: DVE (fused stt), Pool
(two tensor-tensor ops with broadcast r^{u+1} tiles), or PE (extra
matmuls against V with r^{u+1}-scaled C weights, accumulated in the same
PSUM as the main output matmuls).

Host pre/post: u-major permutation of x and y, weight/table precompute.
"""

import sys

for _p in ("/opt/trn_rl_repo", "/root/.axon_site/_ro/trn_rl_repo"):
    if _p not in sys.path:
        sys.path.append(_p)

import numpy as np

N = 256
T = 8192
BATCH = 8
NCORES = 8
K = 16          # chunk length (weight-set count)
NC = T // K     # number of chunks = columns per u-row
CS = 2          # c-slabs for pipelining
NCs = NC // CS

U_DVE = (0, 1, 2, 4, 6, 8, 10, 12, 14)
U_POOL = ()
U_PE = (3, 5, 7, 9, 11, 13, 15)

_cache = {}


def _build_program():
    from concourse import bacc, tile
    from concourse import mybir

    fp32 = mybir.dt.float32
    fp16 = mybir.dt.float16
    Copy = mybir.ActivationFunctionType.Copy
    mult = mybir.AluOpType.mult
    add = mybir.AluOpType.add

    nc = bacc.Bacc(None, target_bir_lowering=False, debug=False)

    # DRAM parameters (per core) - packed for few, large DMAs
    NW2 = len(U_PE) * 4
    xall = nc.declare_dram_parameter("xall", [128, 2 * T], fp16,
                                     isOutput=False)
    wBall = nc.declare_dram_parameter("wBall", [128, 64 * N], fp16,
                                      isOutput=False)
    wCall = nc.declare_dram_parameter("wCall", [128, (64 + NW2) * N], fp16,
                                      isOutput=False)
    tab16 = nc.declare_dram_parameter("tab16", [N, 3 * NC], fp16,
                                      isOutput=False)
    tab32 = nc.declare_dram_parameter("tab32", [N, 1 + K], fp32,
                                      isOutput=False)
    outy = nc.declare_dram_parameter("outy", [N, T], fp16, isOutput=True)

    PL = ("re", "im")
    # wCall col-block index for wc(u,p,nh): (u*2+p)*2+nh ; wc2(i,p,nh): 64+...
    UE_E = tuple(u for u in U_PE if u < 10)   # early wc2 (in wcE pool)
    UE_L = tuple(u for u in U_PE if u >= 10)  # late wc2 (in wcL pool)

    with tile.TileContext(nc) as tc:
        with (
            tc.tile_pool(name="const", bufs=1) as cpool,
            tc.tile_pool(name="m", bufs=1) as mpool,
            tc.tile_pool(name="ck", bufs=1) as ckpool,
            tc.tile_pool(name="bp", bufs=1) as bpool,
            tc.tile_pool(name="w", bufs=2) as wpool,
            tc.tile_pool(name="ys", bufs=3) as ypool,
            tc.tile_pool(name="pa", bufs=1, space="PSUM") as papool,
            tc.tile_pool(name="py", bufs=2, space="PSUM") as pypool,
        ):
            # ---- persistent small tables ----
            cosT, sinT, rkbT, rcolT, rseedT = {}, {}, {}, {}, {}
            tabdma = []
            for mh in range(2):
                t16 = cpool.tile([128, 3 * NC], fp16, name=f"t16_{mh}",
                                 tag=f"t16_{mh}")
                cosT[mh] = t16[:, 0:NC]
                sinT[mh] = t16[:, NC:2 * NC]
                rkbT[mh] = t16[:, 2 * NC:3 * NC]
                t32 = cpool.tile([128, 1 + K], fp32, name=f"t32_{mh}",
                                 tag=f"t32_{mh}")
                rcolT[mh] = t32[:, 0:1]
                rseedT[mh] = t32[:, 1:1 + K]
                tabdma.append((t16, t32, mh))

            m = {}
            for u in range(K):
                for pl in PL:
                    for mh in range(2):
                        m[(u, pl, mh)] = mpool.tile(
                            [128, NC], fp16, name=f"m{u}{pl}{mh}",
                            tag=f"m{u}{pl}{mh}")
            g, V = {}, {}
            for pl in PL:
                for mh in range(2):
                    gt = ckpool.tile([128, NC + 1], fp16, name=f"g{pl}{mh}",
                                     tag=f"g{pl}{mh}")
                    nc.gpsimd.memset(gt[:, 0:1], 0.0)
                    g[(pl, mh)] = gt
                    V[(pl, mh)] = ckpool.tile([128, NC], fp16,
                                              name=f"V{pl}{mh}",
                                              tag=f"V{pl}{mh}")

            wc, wc2 = {}, {}

            # =========== phase 1: A (whole NC), with x+wB pools ===========
            wcE_cm = tc.tile_pool(name="wcE", bufs=1)
            wcE = wcE_cm.__enter__()
            with tc.tile_pool(name="xw", bufs=1) as xwpool:
                xbig = xwpool.tile([128, 2 * T], fp16, name="xbig",
                                   tag="xbig")
                wbbig = xwpool.tile([128, 64 * N], fp16, name="wbbig",
                                    tag="wbbig")
                # early wc pool: wc u<10 + wc2 for UE_E
                wcEt = wcE.tile([128, (40 + 4 * len(UE_E)) * N], fp16,
                                name="wcE", tag="wcE")
                for u in range(10):
                    for p in range(2):
                        for nh in range(2):
                            b0 = ((u * 2 + p) * 2 + nh) * N
                            wc[(u, PL[p], nh)] = wcEt[:, b0:b0 + N]
                for j, u in enumerate(UE_E):
                    for p in range(2):
                        for nh in range(2):
                            b0 = (40 + (j * 2 + p) * 2 + nh) * N
                            wc2[(u, PL[p], nh)] = wcEt[:, b0:b0 + N]

                # fp32 tables first (tiny; the chain stt needs rcolT)
                for t16, t32, mh in tabdma:
                    sl = slice(mh * 128, (mh + 1) * 128)
                    nc.sync.dma_start(out=t32[:], in_=tab32[sl, :])
                # DMA pieces ordered by consumption (first pieces 1-u)
                for up0, up1 in ((0, 1), (1, 2)) + tuple(
                        (i, i + 2) for i in range(2, K, 2)):
                    c0, c1 = up0 * 4 * N, up1 * 4 * N
                    nc.sync.dma_start(out=wbbig[:, c0:c1],
                                      in_=wBall[:, c0:c1])
                    for nh in range(2):
                        x0 = nh * T + up0 * NC
                        x1 = nh * T + up1 * NC
                        nc.sync.dma_start(out=xbig[:, x0:x1],
                                          in_=xall[:, x0:x1])
                    up = up0
                    if up == 2:
                        for t16, _t32, mh in tabdma:
                            sl = slice(mh * 128, (mh + 1) * 128)
                            nc.sync.dma_start(out=t16[:], in_=tab16[sl, :])
                # early wc: wc u<10 blocks are wCall cols [0:40N); wc2 blocks
                # for UE_E start at (64 + idx(U_PE)*4)*N
                nc.sync.dma_start(out=wcEt[:, 0:20 * N],
                                  in_=wCall[:, 0:20 * N])
                nc.sync.dma_start(out=wcEt[:, 20 * N:40 * N],
                                  in_=wCall[:, 20 * N:40 * N])
                if UE_E:
                    i0 = U_PE.index(UE_E[0])
                    s0, s1 = (64 + i0 * 4) * N, (64 + (i0 + len(UE_E)) * 4) * N
                    half = 40 * N + (s1 - s0) // 2
                    nc.sync.dma_start(out=wcEt[:, 40 * N:half],
                                      in_=wCall[:, s0:s0 + (s1 - s0) // 2])
                    nc.sync.dma_start(out=wcEt[:, half:],
                                      in_=wCall[:, s0 + (s1 - s0) // 2:s1])

                # ---- Phase A: whole-NC rows ----
                for u in range(K):
                    ps = {}
                    for pl in PL:
                        p = PL.index(pl)
                        for mh in range(2):
                            p_ = papool.tile([128, NC], fp32,
                                             name=f"pa{pl}{mh}",
                                             tag=f"pa{pl}{mh}")
                            for nh in range(2):
                                b0 = ((u * 2 + p) * 2 + nh) * N
                                nc.tensor.matmul(
                                    p_[:],
                                    wbbig[:, b0 + mh * 128:b0 + mh * 128 + 128],
                                    xbig[:, nh * T + u * NC:
                                         nh * T + (u + 1) * NC],
                                    start=(nh == 0), stop=(nh == 1),
                                )
                            ps[(pl, mh)] = p_
                    for pl in PL:
                        for mh in range(2):
                            dst = m[(u, pl, mh)][:]
                            if u == 0:
                                nc.scalar.activation(dst, ps[(pl, mh)][:],
                                                     Copy)
                            else:
                                nc.vector.scalar_tensor_tensor(
                                    dst, m[(u - 1, pl, mh)][:],
                                    rcolT[mh][:, 0:1], ps[(pl, mh)][:],
                                    mult, add,
                                )

            # =========== phase 2: B + C, slabbed ===========
            with tc.tile_pool(name="wcL", bufs=1) as wcL:
                pass_marker = None
                wcLt = wcL.tile([128, (24 + 4 * len(UE_L)) * N], fp16,
                                name="wcL", tag="wcL")
                for u in range(10, K):
                    for p in range(2):
                        for nh in range(2):
                            b0 = ((u - 10) * 4 + p * 2 + nh) * N
                            wc[(u, PL[p], nh)] = wcLt[:, b0:b0 + N]
                for j, u in enumerate(UE_L):
                    for p in range(2):
                        for nh in range(2):
                            b0 = (24 + (j * 2 + p) * 2 + nh) * N
                            wc2[(u, PL[p], nh)] = wcLt[:, b0:b0 + N]
                # late wc DMAs (hidden under B0 / early C)
                nc.sync.dma_start(out=wcLt[:, 0:12 * N],
                                  in_=wCall[:, 40 * N:52 * N])
                nc.sync.dma_start(out=wcLt[:, 12 * N:24 * N],
                                  in_=wCall[:, 52 * N:64 * N])
                if UE_L:
                    i0 = U_PE.index(UE_L[0])
                    s0 = (64 + i0 * 4) * N
                    s1 = (64 + (i0 + len(UE_L)) * 4) * N
                    nc.sync.dma_start(out=wcLt[:, 24 * N:],
                                      in_=wCall[:, s0:s1])

                ones = wcL.tile([128, NC], fp16, name="ones", tag="ones")
                nc.gpsimd.memset(ones[:], 1.0)
                rsb = {}
                for u in U_POOL:
                    for mh in range(2):
                        t_ = wcL.tile([128, NC], fp16, name=f"rsb{u}{mh}",
                                      tag=f"rsb{u}{mh}")
                        nc.scalar.activation(t_[:], ones[:], Copy,
                                             scale=rseedT[mh][:, u:u + 1])
                        rsb[(u, mh)] = t_

                def phase_b(s):
                    lo, hi = s * NCs, (s + 1) * NCs
                    for mh in range(2):
                        pre = m[(K - 1, "re", mh)][:, lo:hi]
                        pim = m[(K - 1, "im", mh)][:, lo:hi]
                        cs_ = cosT[mh][:, lo:hi]
                        sn_ = sinT[mh][:, lo:hi]
                        a = bpool.tile([128, NCs], fp16, name=f"ba{mh}",
                                       tag=f"ba{mh}")
                        b = bpool.tile([128, NCs], fp16, name=f"bb{mh}",
                                       tag=f"bb{mh}")
                        pr = bpool.tile([128, NCs], fp16, name=f"pr{mh}",
                                        tag=f"pr{mh}")
                        pi = bpool.tile([128, NCs], fp16, name=f"pi{mh}",
                                        tag=f"pi{mh}")
                        nc.vector.tensor_mul(a[:], cs_, pre)
                        nc.vector.tensor_mul(b[:], sn_, pim)
                        nc.vector.tensor_add(pr[:], a[:], b[:])
                        nc.vector.tensor_mul(a[:], cs_, pim)
                        nc.vector.tensor_mul(b[:], sn_, pre)
                        nc.vector.tensor_sub(pi[:], a[:], b[:])
                        for pl, srct in (("re", pr), ("im", pi)):
                            gt = g[(pl, mh)]
                            nc.vector.tensor_tensor_scan(
                                gt[:, lo + 1:hi + 1], rkbT[mh][:, lo:hi],
                                srct[:], gt[:, lo:lo + 1], mult, add,
                            )
                        gre = g[("re", mh)][:, lo:hi]
                        gim = g[("im", mh)][:, lo:hi]
                        nc.vector.tensor_mul(a[:], cs_, gre)
                        nc.vector.tensor_mul(b[:], sn_, gim)
                        nc.vector.tensor_sub(V[("re", mh)][:, lo:hi],
                                             a[:], b[:])
                        nc.vector.tensor_mul(a[:], cs_, gim)
                        nc.vector.tensor_mul(b[:], sn_, gre)
                        nc.vector.tensor_add(V[("im", mh)][:, lo:hi],
                                             a[:], b[:])

                yq = {}

                pyheld = {}

                def prefill_c_u(s, u):
                    # m-part matmuls for a U_PE u; psum held until finish
                    lo, hi = s * NCs, (s + 1) * NCs
                    for mo in range(2):
                        p_ = pypool.tile([128, NCs], fp32, name=f"py{mo}",
                                         tag=f"py{mo}")
                        kk = 0
                        for pl in PL:
                            for mh in range(2):
                                wcs = wc[(u, pl, mh)]
                                nc.tensor.matmul(
                                    p_[:],
                                    wcs[:, mo * 128:(mo + 1) * 128],
                                    m[(u, pl, mh)][:, lo:hi],
                                    start=(kk == 0), stop=False,
                                )
                                kk += 1
                        pyheld[(s, u, mo)] = p_

                def finish_c_u(s, u):
                    lo, hi = s * NCs, (s + 1) * NCs
                    for mo in range(2):
                        p_ = pyheld.pop((s, u, mo))
                        kk = 4
                        for pl in PL:
                            for mh in range(2):
                                w2s = wc2[(u, pl, mh)]
                                nc.tensor.matmul(
                                    p_[:],
                                    w2s[:, mo * 128:(mo + 1) * 128],
                                    V[(pl, mh)][:, lo:hi],
                                    start=False, stop=(kk == 7),
                                )
                                kk += 1
                        _evict_y(s, u, mo, p_)

                def _evict_y(s, u, mo, p_):
                    q, uq = u // 4, u % 4
                    ent = yq.get((s, mo, q))
                    if ent is None:
                        ent = [ypool.tile([128, 4 * NCs], fp16,
                                          name=f"y{mo}", tag=f"y{mo}"), 0]
                        yq[(s, mo, q)] = ent
                    yt = ent[0]
                    nc.scalar.activation(yt[:, uq * NCs:(uq + 1) * NCs],
                                         p_[:], Copy)
                    ent[1] += 1
                    if ent[1] == 4:
                        base = s * T // CS + q * 4 * NCs
                        nc.scalar.dma_start(
                            out=outy[mo * 128:(mo + 1) * 128,
                                     base:base + 4 * NCs],
                            in_=yt[:],
                        )
                        del yq[(s, mo, q)]

                def phase_c_u(s, u):
                    lo, hi = s * NCs, (s + 1) * NCs
                    w_ = {}
                    if u in U_PE:
                        for pl in PL:
                            for mh in range(2):
                                w_[(pl, mh)] = m[(u, pl, mh)][:, lo:hi]
                    else:
                        for pl in PL:
                            for mh in range(2):
                                t_ = wpool.tile([128, NCs], fp16,
                                                name=f"w{pl}{mh}",
                                                tag=f"w{pl}{mh}")
                                if u in U_POOL:
                                    t2 = wcL.tile([128, NCs], fp16,
                                                  name=f"vt{pl}{mh}",
                                                  tag=f"vt{pl}{mh}")
                                    nc.gpsimd.tensor_mul(
                                        t2[:], rsb[(u, mh)][:, lo:hi],
                                        V[(pl, mh)][:, lo:hi])
                                    nc.gpsimd.tensor_add(
                                        t_[:], t2[:],
                                        m[(u, pl, mh)][:, lo:hi])
                                else:
                                    nc.vector.scalar_tensor_tensor(
                                        t_[:], V[(pl, mh)][:, lo:hi],
                                        rseedT[mh][:, u:u + 1],
                                        m[(u, pl, mh)][:, lo:hi],
                                        mult, add,
                                    )
                                w_[(pl, mh)] = t_
                    for mo in range(2):
                        p_ = pypool.tile([128, NCs], fp32, name=f"py{mo}",
                                         tag=f"py{mo}")
                        nmm = 8 if u in U_PE else 4
                        kk = 0
                        for pl in PL:
                            p = PL.index(pl)
                            for mh in range(2):
                                wcs = wc[(u, pl, mh)]
                                nc.tensor.matmul(
                                    p_[:],
                                    wcs[:, mo * 128:(mo + 1) * 128],
                                    w_[(pl, mh)][:],
                                    start=(kk == 0), stop=(kk == nmm - 1),
                                )
                                kk += 1
                        if u in U_PE:
                            for pl in PL:
                                for mh in range(2):
                                    w2s = wc2[(u, pl, mh)]
                                    nc.tensor.matmul(
                                        p_[:],
                                        w2s[:, mo * 128:(mo + 1) * 128],
                                        V[(pl, mh)][:, lo:hi],
                                        start=False, stop=(kk == nmm - 1),
                                    )
                                    kk += 1
                        _evict_y(s, u, mo, p_)

                c_order = (7, 0, 1, 9, 2, 4, 6, 8, 10,
                           11, 12, 14, 13, 15)
                pre0 = (3, 5)
                for u in pre0:
                    prefill_c_u(0, u)
                phase_b(0)
                for u in pre0:
                    finish_c_u(0, u)
                for u in c_order:
                    if u not in pre0:
                        phase_c_u(0, u)
                for u in pre0:
                    prefill_c_u(1, u)
                phase_b(1)
                for u in pre0:
                    finish_c_u(1, u)
                for u in c_order:
                    if u not in pre0:
                        phase_c_u(1, u)
            wcE_cm.__exit__(None, None, None)

    nc.compile()
    return nc


def _host_prep(x, nu_log, theta_log, gamma_log, B_re, B_im, C_re, C_im):
    f64 = np.float64
    r = np.exp(-np.exp(nu_log.astype(f64)))            # [N]
    theta = np.exp(theta_log.astype(f64))              # [N]
    gamma = np.exp(gamma_log.astype(f64))              # [N]
    Bre = B_re.astype(f64) * gamma[:, None]            # [N,N] rows=out m
    Bim = B_im.astype(f64) * gamma[:, None]
    Cre = C_re.astype(f64)
    Cim = C_im.astype(f64)

    u = np.arange(K, dtype=f64)
    cu, su = np.cos(theta[:, None] * u), np.sin(theta[:, None] * u)  # [N,K]

    wBs = np.empty((K * 2 * N, N), dtype=f64)
    wCs = np.empty((K * 2 * N, N), dtype=f64)
    wC2s = np.empty((len(U_PE) * 2 * N, N), dtype=f64)
    rpow = r[:, None] ** (u[None, :] + 1.0)            # [N,K]
    for uu in range(K):
        c_, s_ = cu[:, uu], su[:, uu]
        reB = c_[:, None] * Bre + s_[:, None] * Bim
        imB = c_[:, None] * Bim - s_[:, None] * Bre
        wBs[(2 * uu) * N:(2 * uu + 1) * N] = reB.T
        wBs[(2 * uu + 1) * N:(2 * uu + 2) * N] = imB.T
        reC = (c_[None, :] * Cre - s_[None, :] * Cim).T
        imC = -(s_[None, :] * Cre + c_[None, :] * Cim).T
        wCs[(2 * uu) * N:(2 * uu + 1) * N] = reC
        wCs[(2 * uu + 1) * N:(2 * uu + 2) * N] = imC
        if uu in U_PE:
            i = U_PE.index(uu)
            sc = rpow[:, uu][:, None]
            wC2s[(2 * i) * N:(2 * i + 1) * N] = sc * reC
            wC2s[(2 * i + 1) * N:(2 * i + 2) * N] = sc * imC

    phi = K * theta
    c_ = np.arange(NC, dtype=f64)
    ang = phi[:, None] * c_[None, :]

    def pack_w(ws):
        # rows [(u*2+p)*N + nh*128 : +128] -> col block ((u*2+p)*2+nh)*N
        nb = ws.shape[0] // 128
        return np.concatenate([ws[i * 128:(i + 1) * 128, :]
                               for i in range(nb)], axis=1)

    cosv = np.cos(ang)
    sinv = np.sin(ang)
    rkv = np.broadcast_to((r ** K)[:, None], (N, NC))
    shared = {
        "wBall": pack_w(wBs).astype(np.float16),
        "wCall": np.concatenate(
            [pack_w(wCs), pack_w(wC2s)], axis=1).astype(np.float16),
        "tab16": np.concatenate([cosv, sinv, rkv], axis=1).astype(np.float16),
        "tab32": np.concatenate([r[:, None], rpow], axis=1).astype(np.float32),
    }
    # u-major x: xu[:, u*NC + c] = xT[:, c*K + u]; both nh halves packed
    xus = []
    for b in range(BATCH):
        xT = np.ascontiguousarray(x[b].T).astype(np.float16)   # [N, T]
        xum = xT.reshape(N, NC, K).transpose(0, 2, 1).reshape(N, T)
        xa = np.concatenate([xum[0:128, :], xum[128:256, :]], axis=1)
        xus.append(np.ascontiguousarray(xa))
    return shared, xus


def kernel(x, nu_log, theta_log, gamma_log, B_re, B_im, C_re, C_im,
           _want_trace=False):
    from concourse import bass_utils

    x = np.asarray(x)
    nu_log = np.asarray(nu_log)
    theta_log = np.asarray(theta_log)
    gamma_log = np.asarray(gamma_log)
    B_re, B_im = np.asarray(B_re), np.asarray(B_im)
    C_re, C_im = np.asarray(C_re), np.asarray(C_im)

    if "nc" not in _cache:
        _cache["nc"] = _build_program()
    nc = _cache["nc"]

    shared, xus = _host_prep(x, nu_log, theta_log, gamma_log,
                             B_re, B_im, C_re, C_im)
    in_maps = [dict(shared, xall=xus[i]) for i in range(NCORES)]
    import os
    os.environ["BASS_NEVER_TRACE"] = "1"
    res = bass_utils.run_bass_kernel_spmd(
        nc, in_maps, core_ids=list(range(NCORES)), trace=False,
    )
    _cache["last_result"] = res
    outs = []
    for i in range(NCORES):
        # layout: [n, s, q, uq, coff]; t = (s*NCs+coff)*K + (q*4+uq)
        yu = res.results[i]["outy"]
        yT = (yu.reshape(N, CS, K // 4, 4, NCs)
              .transpose(0, 1, 4, 2, 3).reshape(N, T))
        outs.append(yT.T)
    return np.stack(outs).astype(np.float32)


if __name__ == "__main__":
    rng = np.random.default_rng(0)
    ins = {
        "x": rng.standard_normal((BATCH, T, N), dtype=np.float32),
        "nu_log": rng.standard_normal(N).astype(np.float32),
        "theta_log": rng.standard_normal(N).astype(np.float32),
        "gamma_log": rng.standard_normal(N).astype(np.float32),
        "B_re": rng.standard_normal((N, N), dtype=np.float32) * 0.04,
        "B_im": rng.standard_normal((N, N), dtype=np.float32) * 0.04,
        "C_re": rng.standard_normal((N, N), dtype=np.float32) * 0.06,
        "C_im": rng.standard_normal((N, N), dtype=np.float32) * 0.06,
    }
    out = kernel(**ins)
    print("out", out.shape, out.dtype, np.abs(out).max())


# revision 29
# speedup vs baseline: 1.0382x; 1.0326x over previous
"""LRU (Linear Recurrent Unit) Bass kernel for Trainium2, 8 NeuronCores.

v4.1: chunked-weights formulation. All per-timestep complex rotations are
folded into K per-offset weight matrices (u = t mod K), so the PE does
them for free. Data lives in u-major layout (pos = u*NC + c):

  bt'_u = (diag(e^{-i theta u}) B~)^T x_u      (PE, K weight sets)
  m_u   = r (.) m_{u-1} + bt'_u                (DVE fused stt per u-row)
  chunk carry: W_c = lambda^K W_{c-1} + P_c    (small complex scan, NC cols)
  w_u   = m_u + r^{u+1} (.) V,  V_c = e^{i phi c} G_{c-1}
  y     = ReC^_u w_re - ImC^_u w_im            (PE, K weight sets)

The w-correction is split across engines per u: DVE (fused stt), Pool
(two tensor-tensor ops with broadcast r^{u+1} tiles), or PE (extra
matmuls against V with r^{u+1}-scaled C weights, accumulated in the same
PSUM as the main output matmuls).

Host pre/post: u-major permutation of x and y, weight/table precompute.
"""

import sys

for _p in ("/opt/trn_rl_repo", "/root/.axon_site/_ro/trn_rl_repo"):
    if _p not in sys.path:
        sys.path.append(_p)

import numpy as np

N = 256
T = 8192
BATCH = 8
NCORES = 8
K = 16          # chunk length (weight-set count)
NC = T // K     # number of chunks = columns per u-row
CS = 2          # c-slabs for pipelining
NCs = NC // CS

U_DVE = (0, 1, 2, 4, 6, 8, 10, 12, 14)
U_POOL = ()
U_PE = (3, 5, 7, 9, 11, 13, 15)

_cache = {}


def _build_program():
    from concourse import bacc, tile
    from concourse import mybir

    fp32 = mybir.dt.float32
    fp16 = mybir.dt.float16
    Copy = mybir.ActivationFunctionType.Copy
    mult = mybir.AluOpType.mult
    add = mybir.AluOpType.add

    nc = bacc.Bacc(None, target_bir_lowering=False, debug=False)

    # DRAM parameters (per core) - packed for few, large DMAs
    NW2 = len(U_PE) * 4
    xall = nc.declare_dram_parameter("xall", [128, 2 * T], fp16,
                                     isOutput=False)
    wBall = nc.declare_dram_parameter("wBall", [128, 64 * N], fp16,
                                      isOutput=False)
    wCall = nc.declare_dram_parameter("wCall", [128, (64 + NW2) * N], fp16,
                                      isOutput=False)
    tab16 = nc.declare_dram_parameter("tab16", [N, 3 * NC], fp16,
                                      isOutput=False)
    tab32 = nc.declare_dram_parameter("tab32", [N, 1 + K], fp32,
                                      isOutput=False)
    outy = nc.declare_dram_parameter("outy", [N, T], fp16, isOutput=True)

    PL = ("re", "im")
    # wCall col-block index for wc(u,p,nh): (u*2+p)*2+nh ; wc2(i,p,nh): 64+...
    UE_E = tuple(u for u in U_PE if u < 10)   # early wc2 (in wcE pool)
    UE_L = tuple(u for u in U_PE if u >= 10)  # late wc2 (in wcL pool)

    with tile.TileContext(nc) as tc:
        with (
            tc.tile_pool(name="const", bufs=1) as cpool,
            tc.tile_pool(name="m", bufs=1) as mpool,
            tc.tile_pool(name="ck", bufs=1) as ckpool,
            tc.tile_pool(name="bp", bufs=1) as bpool,
            tc.tile_pool(name="w", bufs=2) as wpool,
            tc.tile_pool(name="ys", bufs=3) as ypool,
            tc.tile_pool(name="pa", bufs=1, space="PSUM") as papool,
            tc.tile_pool(name="py", bufs=2, space="PSUM") as pypool,
        ):
            # ---- persistent small tables ----
            cosT, sinT, rkbT, rcolT, rseedT = {}, {}, {}, {}, {}
            tabdma = []
            for mh in range(2):
                t16 = cpool.tile([128, 3 * NC], fp16, name=f"t16_{mh}",
                                 tag=f"t16_{mh}")
                cosT[mh] = t16[:, 0:NC]
                sinT[mh] = t16[:, NC:2 * NC]
                rkbT[mh] = t16[:, 2 * NC:3 * NC]
                t32 = cpool.tile([128, 1 + K], fp32, name=f"t32_{mh}",
                                 tag=f"t32_{mh}")
                rcolT[mh] = t32[:, 0:1]
                rseedT[mh] = t32[:, 1:1 + K]
                tabdma.append((t16, t32, mh))

            m = {}
            for u in range(K):
                for pl in PL:
                    for mh in range(2):
                        m[(u, pl, mh)] = mpool.tile(
                            [128, NC], fp16, name=f"m{u}{pl}{mh}",
                            tag=f"m{u}{pl}{mh}")
            g, V = {}, {}
            for pl in PL:
                for mh in range(2):
                    gt = ckpool.tile([128, NC + 1], fp16, name=f"g{pl}{mh}",
                                     tag=f"g{pl}{mh}")
                    nc.gpsimd.memset(gt[:, 0:1], 0.0)
                    g[(pl, mh)] = gt
                    V[(pl, mh)] = ckpool.tile([128, NC], fp16,
                                              name=f"V{pl}{mh}",
                                              tag=f"V{pl}{mh}")

            wc, wc2 = {}, {}

            # =========== phase 1: A (whole NC), with x+wB pools ===========
            wcE_cm = tc.tile_pool(name="wcE", bufs=1)
            wcE = wcE_cm.__enter__()
            with tc.tile_pool(name="xw", bufs=1) as xwpool:
                xbig = xwpool.tile([128, 2 * T], fp16, name="xbig",
                                   tag="xbig")
                wbbig = xwpool.tile([128, 64 * N], fp16, name="wbbig",
                                    tag="wbbig")
                # early wc pool: wc u<10 + wc2 for UE_E
                wcEt = wcE.tile([128, (40 + 4 * len(UE_E)) * N], fp16,
                                name="wcE", tag="wcE")
                for u in range(10):
                    for p in range(2):
                        for nh in range(2):
                            b0 = ((u * 2 + p) * 2 + nh) * N
                            wc[(u, PL[p], nh)] = wcEt[:, b0:b0 + N]
                for j, u in enumerate(UE_E):
                    for p in range(2):
                        for nh in range(2):
                            b0 = (40 + (j * 2 + p) * 2 + nh) * N
                            wc2[(u, PL[p], nh)] = wcEt[:, b0:b0 + N]

                # fp32 tables first (tiny; the chain stt needs rcolT)
                for t16, t32, mh in tabdma:
                    sl = slice(mh * 128, (mh + 1) * 128)
                    nc.sync.dma_start(out=t32[:], in_=tab32[sl, :])
                # DMA pieces ordered by consumption (first pieces 1-u)
                for up0, up1 in ((0, 1), (1, 2)) + tuple(
                        (i, i + 2) for i in range(2, K, 2)):
                    c0, c1 = up0 * 4 * N, up1 * 4 * N
                    nc.sync.dma_start(out=wbbig[:, c0:c1],
                                      in_=wBall[:, c0:c1])
                    for nh in range(2):
                        x0 = nh * T + up0 * NC
                        x1 = nh * T + up1 * NC
                        nc.sync.dma_start(out=xbig[:, x0:x1],
                                          in_=xall[:, x0:x1])
                    up = up0
                    if up == 2:
                        for t16, _t32, mh in tabdma:
                            sl = slice(mh * 128, (mh + 1) * 128)
                            nc.sync.dma_start(out=t16[:], in_=tab16[sl, :])
                # early wc: wc u<10 blocks are wCall cols [0:40N); wc2 blocks
                # for UE_E start at (64 + idx(U_PE)*4)*N
                nc.sync.dma_start(out=wcEt[:, 0:20 * N],
                                  in_=wCall[:, 0:20 * N])
                nc.sync.dma_start(out=wcEt[:, 20 * N:40 * N],
                                  in_=wCall[:, 20 * N:40 * N])
                if UE_E:
                    i0 = U_PE.index(UE_E[0])
                    s0, s1 = (64 + i0 * 4) * N, (64 + (i0 + len(UE_E)) * 4) * N
                    half = 40 * N + (s1 - s0) // 2
                    nc.sync.dma_start(out=wcEt[:, 40 * N:half],
                                      in_=wCall[:, s0:s0 + (s1 - s0) // 2])
                    nc.sync.dma_start(out=wcEt[:, half:],
                                      in_=wCall[:, s0 + (s1 - s0) // 2:s1])

                # ---- Phase A: whole-NC rows ----
                for u in range(K):
                    ps = {}
                    for pl in PL:
                        p = PL.index(pl)
                        for mh in range(2):
                            p_ = papool.tile([128, NC], fp32,
                                             name=f"pa{pl}{mh}",
                                             tag=f"pa{pl}{mh}")
                            for nh in range(2):
                                b0 = ((u * 2 + p) * 2 + nh) * N
                                nc.tensor.matmul(
                                    p_[:],
                                    wbbig[:, b0 + mh * 128:b0 + mh * 128 + 128],
                                    xbig[:, nh * T + u * NC:
                                         nh * T + (u + 1) * NC],
                                    start=(nh == 0), stop=(nh == 1),
                                )
                            ps[(pl, mh)] = p_
                    for pl in PL:
                        for mh in range(2):
                            dst = m[(u, pl, mh)][:]
                            if u == 0:
                                nc.scalar.activation(dst, ps[(pl, mh)][:],
                                                     Copy)
                            else:
                                nc.vector.scalar_tensor_tensor(
                                    dst, m[(u - 1, pl, mh)][:],
                                    rcolT[mh][:, 0:1], ps[(pl, mh)][:],
                                    mult, add,
                                )

            # =========== phase 2: B + C, slabbed ===========
            with tc.tile_pool(name="wcL", bufs=1) as wcL:
                pass_marker = None
                wcLt = wcL.tile([128, (24 + 4 * len(UE_L)) * N], fp16,
                                name="wcL", tag="wcL")
                for u in range(10, K):
                    for p in range(2):
                        for nh in range(2):
                            b0 = ((u - 10) * 4 + p * 2 + nh) * N
                            wc[(u, PL[p], nh)] = wcLt[:, b0:b0 + N]
                for j, u in enumerate(UE_L):
                    for p in range(2):
                        for nh in range(2):
                            b0 = (24 + (j * 2 + p) * 2 + nh) * N
                            wc2[(u, PL[p], nh)] = wcLt[:, b0:b0 + N]
                # late wc DMAs (hidden under B0 / early C)
                nc.sync.dma_start(out=wcLt[:, 0:12 * N],
                                  in_=wCall[:, 40 * N:52 * N])
                nc.sync.dma_start(out=wcLt[:, 12 * N:24 * N],
                                  in_=wCall[:, 52 * N:64 * N])
                if UE_L:
                    i0 = U_PE.index(UE_L[0])
                    s0 = (64 + i0 * 4) * N
                    s1 = (64 + (i0 + len(UE_L)) * 4) * N
                    nc.sync.dma_start(out=wcLt[:, 24 * N:],
                                      in_=wCall[:, s0:s1])

                ones = wcL.tile([128, NC], fp16, name="ones", tag="ones")
                nc.gpsimd.memset(ones[:], 1.0)
                rsb = {}
                for u in U_POOL:
                    for mh in range(2):
                        t_ = wcL.tile([128, NC], fp16, name=f"rsb{u}{mh}",
                                      tag=f"rsb{u}{mh}")
                        nc.scalar.activation(t_[:], ones[:], Copy,
                                             scale=rseedT[mh][:, u:u + 1])
                        rsb[(u, mh)] = t_

                def phase_b(s):
                    lo, hi = s * NCs, (s + 1) * NCs
                    for mh in range(2):
                        pre = m[(K - 1, "re", mh)][:, lo:hi]
                        pim = m[(K - 1, "im", mh)][:, lo:hi]
                        cs_ = cosT[mh][:, lo:hi]
                        sn_ = sinT[mh][:, lo:hi]
                        a = bpool.tile([128, NCs], fp16, name=f"ba{mh}",
                                       tag=f"ba{mh}")
                        b = bpool.tile([128, NCs], fp16, name=f"bb{mh}",
                                       tag=f"bb{mh}")
                        pr = bpool.tile([128, NCs], fp16, name=f"pr{mh}",
                                        tag=f"pr{mh}")
                        pi = bpool.tile([128, NCs], fp16, name=f"pi{mh}",
                                        tag=f"pi{mh}")
                        nc.vector.tensor_mul(a[:], cs_, pre)
                        nc.vector.tensor_mul(b[:], sn_, pim)
                        nc.vector.tensor_add(pr[:], a[:], b[:])
                        nc.vector.tensor_mul(a[:], cs_, pim)
                        nc.vector.tensor_mul(b[:], sn_, pre)
                        nc.vector.tensor_sub(pi[:], a[:], b[:])
                        for pl, srct in (("re", pr), ("im", pi)):
                            gt = g[(pl, mh)]
                            nc.vector.tensor_tensor_scan(
                                gt[:, lo + 1:hi + 1], rkbT[mh][:, lo:hi],
                                srct[:], gt[:, lo:lo + 1], mult, add,
                            )
                        gre = g[("re", mh)][:, lo:hi]
                        gim = g[("im", mh)][:, lo:hi]
                        nc.vector.tensor_mul(a[:], cs_, gre)
                        nc.vector.tensor_mul(b[:], sn_, gim)
                        nc.vector.tensor_sub(V[("re", mh)][:, lo:hi],
                                             a[:], b[:])
                        nc.vector.tensor_mul(a[:], cs_, gim)
                        nc.vector.tensor_mul(b[:], sn_, gre)
                        nc.vector.tensor_add(V[("im", mh)][:, lo:hi],
                                             a[:], b[:])

                yq = {}

                pyheld = {}

                def prefill_c_u(s, u):
                    # m-part matmuls for a U_PE u; psum held until finish
                    lo, hi = s * NCs, (s + 1) * NCs
                    for mo in range(2):
                        p_ = pypool.tile([128, NCs], fp32, name=f"py{mo}",
                                         tag=f"py{mo}")
                        kk = 0
                        for pl in PL:
                            for mh in range(2):
                                wcs = wc[(u, pl, mh)]
                                nc.tensor.matmul(
                                    p_[:],
                                    wcs[:, mo * 128:(mo + 1) * 128],
                                    m[(u, pl, mh)][:, lo:hi],
                                    start=(kk == 0), stop=False,
                                )
                                kk += 1
                        pyheld[(s, u, mo)] = p_

                def finish_c_u(s, u):
                    lo, hi = s * NCs, (s + 1) * NCs
                    for mo in range(2):
                        p_ = pyheld.pop((s, u, mo))
                        kk = 4
                        for pl in PL:
                            for mh in range(2):
                                w2s = wc2[(u, pl, mh)]
                                nc.tensor.matmul(
                                    p_[:],
                                    w2s[:, mo * 128:(mo + 1) * 128],
                                    V[(pl, mh)][:, lo:hi],
                                    start=False, stop=(kk == 7),
                                )
                                kk += 1
                        _evict_y(s, u, mo, p_)

                def _evict_y(s, u, mo, p_):
                    q, uq = u // 4, u % 4
                    ent = yq.get((s, mo, q))
                    if ent is None:
                        ent = [ypool.tile([128, 4 * NCs], fp16,
                                          name=f"y{mo}", tag=f"y{mo}"), 0]
                        yq[(s, mo, q)] = ent
                    yt = ent[0]
                    nc.scalar.activation(yt[:, uq * NCs:(uq + 1) * NCs],
                                         p_[:], Copy)
                    ent[1] += 1
                    if ent[1] == 4:
                        base = s * T // CS + q * 4 * NCs
                        nc.scalar.dma_start(
                            out=outy[mo * 128:(mo + 1) * 128,
                                     base:base + 4 * NCs],
                            in_=yt[:],
                        )
                        del yq[(s, mo, q)]

                def phase_c_u(s, u):
                    lo, hi = s * NCs, (s + 1) * NCs
                    w_ = {}
                    if u in U_PE:
                        for pl in PL:
                            for mh in range(2):
                                w_[(pl, mh)] = m[(u, pl, mh)][:, lo:hi]
                    else:
                        for pl in PL:
                            for mh in range(2):
                                t_ = wpool.tile([128, NCs], fp16,
                                                name=f"w{pl}{mh}",
                                                tag=f"w{pl}{mh}")
                                if u in U_POOL:
                                    t2 = wcL.tile([128, NCs], fp16,
                                                  name=f"vt{pl}{mh}",
                                                  tag=f"vt{pl}{mh}")
                                    nc.gpsimd.tensor_mul(
                                        t2[:], rsb[(u, mh)][:, lo:hi],
                                        V[(pl, mh)][:, lo:hi])
                                    nc.gpsimd.tensor_add(
                                        t_[:], t2[:],
                                        m[(u, pl, mh)][:, lo:hi])
                                else:
                                    nc.vector.scalar_tensor_tensor(
                                        t_[:], V[(pl, mh)][:, lo:hi],
                                        rseedT[mh][:, u:u + 1],
                                        m[(u, pl, mh)][:, lo:hi],
                                        mult, add,
                                    )
                                w_[(pl, mh)] = t_
                    for mo in range(2):
                        p_ = pypool.tile([128, NCs], fp32, name=f"py{mo}",
                                         tag=f"py{mo}")
                        nmm = 8 if u in U_PE else 4
                        kk = 0
                        for pl in PL:
                            p = PL.index(pl)
                            for mh in range(2):
                                wcs = wc[(u, pl, mh)]
                                nc.tensor.matmul(
                                    p_[:],
                                    wcs[:, mo * 128:(mo + 1) * 128],
                                    w_[(pl, mh)][:],
                                    start=(kk == 0), stop=(kk == nmm - 1),
                                )
                                kk += 1
                        if u in U_PE:
                            for pl in PL:
                                for mh in range(2):
                                    w2s = wc2[(u, pl, mh)]
                                    nc.tensor.matmul(
                                        p_[:],
                                        w2s[:, mo * 128:(mo + 1) * 128],
                                        V[(pl, mh)][:, lo:hi],
                                        start=False, stop=(kk == nmm - 1),
                                    )
                                    kk += 1
                        _evict_y(s, u, mo, p_)

                c_order = (5, 0, 1, 7, 2, 4, 9, 6, 8,
                           11, 10, 13, 12, 15, 14)
                pre0 = (3,)
                for u in pre0:
                    prefill_c_u(0, u)
                phase_b(0)
                for u in pre0:
                    finish_c_u(0, u)
                for u in c_order:
                    if u not in pre0:
                        phase_c_u(0, u)
                for u in pre0:
                    prefill_c_u(1, u)
                phase_b(1)
                for u in pre0:
                    finish_c_u(1, u)
                for u in c_order:
                    if u not in pre0:
                        phase_c_u(1, u)
            wcE_cm.__exit__(None, None, None)

    nc.compile()
    return nc


def _host_prep(x, nu_log, theta_log, gamma_log, B_re, B_im, C_re, C_im):
    f64 = np.float64
    r = np.exp(-np.exp(nu_log.astype(f64)))            # [N]
    theta = np.exp(theta_log.astype(f64))              # [N]
    gamma = np.exp(gamma_log.astype(f64))              # [N]
    Bre = B_re.astype(f64) * gamma[:, None]            # [N,N] rows=out m
    Bim = B_im.astype(f64) * gamma[:, None]
    Cre = C_re.astype(f64)
    Cim = C_im.astype(f64)

    u = np.arange(K, dtype=f64)
    cu, su = np.cos(theta[:, None] * u), np.sin(theta[:, None] * u)  # [N,K]

    wBs = np.empty((K * 2 * N, N), dtype=f64)
    wCs = np.empty((K * 2 * N, N), dtype=f64)
    wC2s = np.empty((len(U_PE) * 2 * N, N), dtype=f64)
    rpow = r[:, None] ** (u[None, :] + 1.0)            # [N,K]
    for uu in range(K):
        c_, s_ = cu[:, uu], su[:, uu]
        reB = c_[:, None] * Bre + s_[:, None] * Bim
        imB = c_[:, None] * Bim - s_[:, None] * Bre
        wBs[(2 * uu) * N:(2 * uu + 1) * N] = reB.T
        wBs[(2 * uu + 1) * N:(2 * uu + 2) * N] = imB.T
        reC = (c_[None, :] * Cre - s_[None, :] * Cim).T
        imC = -(s_[None, :] * Cre + c_[None, :] * Cim).T
        wCs[(2 * uu) * N:(2 * uu + 1) * N] = reC
        wCs[(2 * uu + 1) * N:(2 * uu + 2) * N] = imC
        if uu in U_PE:
            i = U_PE.index(uu)
            sc = rpow[:, uu][:, None]
            wC2s[(2 * i) * N:(2 * i + 1) * N] = sc * reC
            wC2s[(2 * i + 1) * N:(2 * i + 2) * N] = sc * imC

    phi = K * theta
    c_ = np.arange(NC, dtype=f64)
    ang = phi[:, None] * c_[None, :]

    def pack_w(ws):
        # rows [(u*2+p)*N + nh*128 : +128] -> col block ((u*2+p)*2+nh)*N
        nb = ws.shape[0] // 128
        return np.concatenate([ws[i * 128:(i + 1) * 128, :]
                               for i in range(nb)], axis=1)

    cosv = np.cos(ang)
    sinv = np.sin(ang)
    rkv = np.broadcast_to((r ** K)[:, None], (N, NC))
    shared = {
        "wBall": pack_w(wBs).astype(np.float16),
        "wCall": np.concatenate(
            [pack_w(wCs), pack_w(wC2s)], axis=1).astype(np.float16),
        "tab16": np.concatenate([cosv, sinv, rkv], axis=1).astype(np.float16),
        "tab32": np.concatenate([r[:, None], rpow], axis=1).astype(np.float32),
    }
    # u-major x: xu[:, u*NC + c] = xT[:, c*K + u]; both nh halves packed
    xus = []
    for b in range(BATCH):
        xT = np.ascontiguousarray(x[b].T).astype(np.float16)   # [N, T]
        xum = xT.reshape(N, NC, K).transpose(0, 2, 1).reshape(N, T)
        xa = np.concatenate([xum[0:128, :], xum[128:256, :]], axis=1)
        xus.append(np.ascontiguousarray(xa))
    return shared, xus


def kernel(x, nu_log, theta_log, gamma_log, B_re, B_im, C_re, C_im,
           _want_trace=False):
    from concourse import bass_utils

    x = np.asarray(x)
    nu_log = np.asarray(nu_log)
    theta_log = np.asarray(theta_log)
    gamma_log = np.asarray(gamma_log)
    B_re, B_im = np.asarray(B_re), np.asarray(B_im)
    C_re, C_im = np.asarray(C_re), np.asarray(C_im)

    if "nc" not in _cache:
        _cache["nc"] = _build_program()
    nc = _cache["nc"]

    shared, xus = _host_prep(x, nu_log, theta_log, gamma_log,
                             B_re, B_im, C_re, C_im)
    in_maps = [dict(shared, xall=xus[i]) for i in range(NCORES)]
    import os
    os.environ["BASS_NEVER_TRACE"] = "1"
    res = bass_utils.run_bass_kernel_spmd(
        nc, in_maps, core_ids=list(range(NCORES)), trace=False,
    )
    _cache["last_result"] = res
    outs = []
    for i in range(NCORES):
        # layout: [n, s, q, uq, coff]; t = (s*NCs+coff)*K + (q*4+uq)
        yu = res.results[i]["outy"]
        yT = (yu.reshape(N, CS, K // 4, 4, NCs)
              .transpose(0, 1, 4, 2, 3).reshape(N, T))
        outs.append(yT.T)
    return np.stack(outs).astype(np.float32)


if __name__ == "__main__":
    rng = np.random.default_rng(0)
    ins = {
        "x": rng.standard_normal((BATCH, T, N), dtype=np.float32),
        "nu_log": rng.standard_normal(N).astype(np.float32),
        "theta_log": rng.standard_normal(N).astype(np.float32),
        "gamma_log": rng.standard_normal(N).astype(np.float32),
        "B_re": rng.standard_normal((N, N), dtype=np.float32) * 0.04,
        "B_im": rng.standard_normal((N, N), dtype=np.float32) * 0.04,
        "C_re": rng.standard_normal((N, N), dtype=np.float32) * 0.06,
        "C_im": rng.standard_normal((N, N), dtype=np.float32) * 0.06,
    }
    out = kernel(**ins)
    print("out", out.shape, out.dtype, np.abs(out).max())
